# revision 34
# baseline (speedup 1.0000x reference)
"""Trainium2 Bass kernel for nn_GCN1PoolNorm: 3-layer GCN + shared BatchNorm +
global max pool + MLP head, SPMD across 8 NeuronCores.

Self-contained: takes FULL inputs, returns FULL output [N_GRAPHS, N_CLASSES].

Design (per core = one 1/8 dst-shard of nodes) — SPARSE gather + one-hot matmul:
- Node table h_tilde = act * dis lives in Shared DRAM as [n_nodes, 64] fp32
  (256B rows — the dma_gather element granularity), AllGather-published per
  layer.
- Per layer, each core gathers the h_tilde rows of its incident edges' src
  nodes with dma_gather (SWDGE, ~0.34ns/descriptor gen + 256B/desc transfer).
  int16 gather indices cap the table at 32768 rows, so edges are split into a
  lo run (src < 32768) and a hi run (gathered from table[32768:]), each
  dst-sorted and chunked to fit SBUF.
- Aggregation agg[dst] = sum_e h_tilde[src_e] runs as a per-128-edge-block
  matmul: psum[64, tile] += M_block.T @ O_block, where M_block [128, 64] is
  the gathered (bf16-converted) block and O_block [128, 128] fp8 is a
  host-built one-hot edge->dst_local matrix. Exact; PSUM accumulation handles
  duplicate dsts. dis[dst] factors out of the sum and is applied after.
- Self-loops are extra edges (src = dst). lo and hi passes accumulate into an
  SBUF agg buffer (copy, then add).
- Downstream per node tile (as v1): U.T = agg * dis_rep; Z.T = W.T @ U.T;
  BN stats via ACT accum_out; stats AllReduce; BN affine+relu fused; * dis;
  PE transpose; DMA to table shard; AllGather.
- Pooling: graphs align exactly to cores; free-axis reduce_max segments;
  MLP head feat-major; out [gpc, 10] per core, host concatenates.
"""
import numpy as np
import ml_dtypes

from concourse import bacc, mybir, tile
from concourse.bass_utils import run_bass_kernel_spmd
from concourse.masks import make_identity

f32 = mybir.dt.float32
bf16 = mybir.dt.bfloat16
fp8 = mybir.dt.float8e4
i16 = mybir.dt.int16

N_CORES = 8
ABLATE = set()   # sim-only ablation knobs ("coll", "gather", "aggmm")
P = 128          # partition / block quantum
D = 64           # feature dim
HALF = 32768     # int16 gather index limit -> lo/hi table split
CC = 16          # gather chunk columns (CC*128 edge slots per chunk)
GC = 8           # columns per dma_gather call (1024 idx = SWDGE ring limit)
OG = 32          # one-hot matrices per O stream group
BN_EPS = 1e-5


# ---------------------------------------------------------------- host prep

def _prep(x, edge_index, batch, n_classes):
    n_nodes = x.shape[0]
    n_graphs = int(batch.max()) + 1
    assert n_nodes % N_CORES == 0
    nsh = n_nodes // N_CORES                    # nodes per core
    ntile = (nsh + P - 1) // P                  # node tiles per core
    tsz = [min(P, nsh - t * P) for t in range(ntile)]

    src_all = np.asarray(edge_index[0], np.int64)
    dst_all = np.asarray(edge_index[1], np.int64)
    deg = np.bincount(dst_all, minlength=n_nodes).astype(np.int64)

    RW = 512
    RNG = (nsh + RW - 1) // RW                  # 512-node dst ranges per core

    # ---- per-core edge lists, sorted by (dst_range, src_half, dst, src):
    # each (range, half) group is a contiguous slot run so the gather
    # stream is a single monotone sequence of chunks per layer
    edges = []                                  # (s, dl, grp) per core
    for c in range(N_CORES):
        m = (dst_all // nsh) == c
        s = src_all[m]
        dl = dst_all[m] - c * nsh
        # self loops
        s = np.concatenate([s, np.arange(c * nsh, (c + 1) * nsh)])
        dl = np.concatenate([dl, np.arange(nsh)])
        hi = (s >= HALF).astype(np.int64)
        grp = (dl // RW) * 2 + hi
        order = np.lexsort((s, dl, grp))
        s, dl, grp = s[order], dl[order], grp[order]
        edges.append((s, dl, grp))

    # SPMD = one program for all cores: pad every (range, half) group to
    # the max block count over cores; pad slots gather table row 0 of the
    # group's half and are masked by all-zero O columns. Schedules are the
    # union over cores; a core lacking a (group, tile, block) gets an
    # all-zero O matrix (adds 0 to psum).
    NGRP = RNG * 2
    gcols = np.zeros(NGRP, np.int64)            # block columns per group
    for c in range(N_CORES):
        cnt = np.bincount(edges[c][2], minlength=NGRP)
        gcols = np.maximum(gcols, (cnt + P - 1) // P)
    gbase = np.concatenate([[0], np.cumsum(gcols)])  # group -> base col
    SC = int(gbase[-1])

    keys = []
    per_edge = []
    for c in range(N_CORES):
        s, dl, grp = edges[c]
        # slot within group run
        gstart = np.searchsorted(grp, np.arange(NGRP))
        slot = gbase[grp] * P + (np.arange(s.shape[0]) - gstart[grp])
        b = slot // P
        t = dl // P
        hi = grp & 1
        # mm issue order: tile-major, each tile's lo mms then hi mms — the
        # tile's matmuls are one contiguous PSUM accumulation group; chunk
        # consumption steps back at most one range's group span
        key = ((t * 2 + hi) << 24) | b
        keys.append(key)
        per_edge.append((s, dl, slot))
    uk = np.unique(np.concatenate(keys))
    nmm = int(uk.shape[0])
    uk_t = uk >> 25
    uk_b = uk & ((1 << 24) - 1)
    # sched[t] = ordered (mi, b) list (lo blocks then hi blocks)
    sched = [[] for _ in range(ntile)]
    for mi in range(nmm):
        sched[int(uk_t[mi])].append((mi, int(uk_b[mi])))

    # gather chunks: (base_col, ncols, is_hi), never spanning a group
    chunks = []
    for g in range(NGRP):
        for c0 in range(0, int(gcols[g]), CC):
            chunks.append((int(gbase[g]) + c0,
                           int(min(CC, gcols[g] - c0)), g & 1))

    NG = (nmm + OG - 1) // OG
    idx_reps, Ots = [], []
    for c in range(N_CORES):
        s, dl, slot = per_edge[c]
        hi_e = s >= HALF
        # gather index array, 16-wrapped and replicated to 128 partitions;
        # pad slots point at table row 0 (junk, masked by zero O columns)
        idx_flat = np.zeros(SC * P, np.int16)
        idx_flat[slot[~hi_e]] = s[~hi_e].astype(np.int16)
        idx_flat[slot[hi_e]] = (s[hi_e] - HALF).astype(np.int16)
        wrap = idx_flat.reshape(SC * 8, 16).T            # [16, SC*8]
        idx_reps.append(np.tile(wrap, (8, 1)))           # [128, SC*8]

        inv = np.searchsorted(uk, keys[c])
        O = np.zeros((nmm, P, P), np.uint8)
        O[inv, slot % P, dl % P] = 1
        # O stream layout: [NG, 128, OG, 128] fp8, group g col j = O[g*OG+j]
        Ot = np.zeros((NG, P, OG, P), ml_dtypes.float8_e4m3)
        Of = O.astype(ml_dtypes.float8_e4m3)
        for g in range(NG):
            k = min(OG, nmm - g * OG)
            Ot[g, :, :k, :] = Of[g * OG:g * OG + k].transpose(1, 0, 2)
        Ots.append(Ot)
        del O, Of

    # deg layouts (fp32)
    deg_pt = np.zeros((N_CORES, P, ntile), np.float32)
    deg_row = np.zeros((N_CORES, 1, nsh), np.float32)
    for c in range(N_CORES):
        dsh = deg[c * nsh:(c + 1) * nsh].astype(np.float32)
        deg_row[c, 0, :] = dsh
        for tt in range(ntile):
            deg_pt[c, :tsz[tt], tt] = dsh[tt * P:tt * P + tsz[tt]]

    # pooling segments (identical across cores required for SPMD)
    gb = np.searchsorted(batch, np.arange(n_graphs + 1))
    gpc = n_graphs // N_CORES
    loc0 = gb[:gpc + 1].copy()
    for c in range(N_CORES):
        locc = gb[c * gpc:(c + 1) * gpc + 1] - c * nsh
        assert np.array_equal(locc, loc0), "graph pattern must match across cores"
    pool_segs = []
    for tt in range(ntile):
        a, bb = tt * P, tt * P + tsz[tt]
        for g in range(gpc):
            ss, ee = max(a, int(loc0[g])), min(bb, int(loc0[g + 1]))
            if ss < ee:
                pool_segs.append((tt, ss - a, ee - a, g))

    cfg = dict(n_nodes=n_nodes, nsh=nsh, ntile=ntile, tsz=tsz,
               pool_segs=pool_segs, gpc=gpc, n_classes=n_classes,
               n_graphs=n_graphs, SC=SC, NG=NG, chunks=chunks, sched=sched)
    data = dict(idx_rep=idx_reps, Ot=Ots, deg_pt=deg_pt, deg_row=deg_row)
    return cfg, data


# ---------------------------------------------------------------- device build

def _build(cfg, reps=1):
    nsh, ntile, tsz = cfg["nsh"], cfg["ntile"], cfg["tsz"]
    ncls, gpc = cfg["n_classes"], cfg["gpc"]
    n_nodes = cfg["n_nodes"]
    SC, NG = cfg["SC"], cfg["NG"]
    nshp = ntile * P
    RW = 512

    sched0 = cfg["sched"]

    R = (nsh + RW - 1) // RW
    rsz = [min(RW, nsh - r * RW) for r in range(R)]

    nc = bacc.Bacc(trn_type="TRN2", target_bir_lowering=False, debug=False,
                   num_devices=N_CORES, num_swdge_queues=4)

    x_sh = nc.dram_tensor("x_sh", [nsh, D], f32, kind="ExternalInput").ap()
    idx_in = nc.dram_tensor("idx", [P, SC * 8], i16, kind="ExternalInput").ap()
    O_in = nc.dram_tensor("O", [NG, P, OG, P], fp8, kind="ExternalInput").ap()
    deg_pt = nc.dram_tensor("deg_pt", [P, ntile], f32, kind="ExternalInput").ap()
    deg_row = nc.dram_tensor("deg_row", [1, nsh], f32, kind="ExternalInput").ap()
    Ws = [nc.dram_tensor(f"W{i}", [D, D], bf16, kind="ExternalInput").ap()
          for i in (1, 2, 3)]
    gamma = nc.dram_tensor("gamma", [D, 1], f32, kind="ExternalInput").ap()
    beta = nc.dram_tensor("beta", [D, 1], f32, kind="ExternalInput").ap()
    lin1w = nc.dram_tensor("lin1w", [D, D], bf16, kind="ExternalInput").ap()
    lin1b = nc.dram_tensor("lin1b", [D, 1], f32, kind="ExternalInput").ap()
    lin2w = nc.dram_tensor("lin2w", [D, ncls], bf16, kind="ExternalInput").ap()
    lin2b = nc.dram_tensor("lin2b", [ncls, 1], f32, kind="ExternalInput").ap()
    out = nc.dram_tensor("out", [gpc, ncls], f32, kind="ExternalOutput").ap()

    table = nc.dram_tensor("table", [n_nodes, D], f32, addr_space="Shared").ap()
    tshard = nc.dram_tensor("tshard", [nsh, D], f32).ap()
    stats_in = nc.dram_tensor("stats_in", [D, 2], f32).ap()
    stats_out = nc.dram_tensor("stats_out", [D, 2], f32,
                               addr_space="Shared").ap()

    with tile.TileContext(nc) as tc:
        with (
            tc.tile_pool(name="const", bufs=1) as cpool,
            tc.tile_pool(name="gath", bufs=8) as gpool,
            tc.tile_pool(name="gbf", bufs=8) as gbpool,
            tc.tile_pool(name="obuf", bufs=6) as opool,
            tc.tile_pool(name="work", bufs=3) as wpool,
            tc.tile_pool(name="psagg", bufs=3, space="PSUM") as ps_agg,
            tc.tile_pool(name="psz", bufs=2, space="PSUM") as ps_z,
            tc.tile_pool(name="pstr", bufs=2, space="PSUM") as ps_tr,
        ):
            # ---- residents
            idx_res = cpool.tile([P, SC * 8], i16)
            nc.sync.dma_start(out=idx_res[:], in_=idx_in[:])
            dis_pt = cpool.tile([P, ntile], f32)
            dis_rep = cpool.tile([D, nshp], f32)
            zbuf = cpool.tile([D, nshp], bf16)
            act3 = zbuf
            sums = cpool.tile([D, R], f32)
            sums2 = cpool.tile([D, R], f32)
            W_sb = [cpool.tile([D, D], bf16, tag=f"W{i}", name=f"W{i}_sb")
                    for i in range(3)]
            for i in range(3):
                nc.sync.dma_start(out=W_sb[i][:], in_=Ws[i][:])
            gamma_sb = cpool.tile([D, 1], f32, tag="gamma")
            beta_sb = cpool.tile([D, 1], f32, tag="beta")
            nc.sync.dma_start(out=gamma_sb[:], in_=gamma[:])
            nc.sync.dma_start(out=beta_sb[:], in_=beta[:])
            l1w_sb = cpool.tile([D, D], bf16, tag="l1w")
            l1b_sb = cpool.tile([D, 1], f32, tag="l1b")
            l2w_sb = cpool.tile([D, ncls], bf16, tag="l2w")
            l2b_sb = cpool.tile([ncls, 1], f32, tag="l2b")
            nc.sync.dma_start(out=l1w_sb[:], in_=lin1w[:])
            nc.sync.dma_start(out=l1b_sb[:], in_=lin1b[:])
            nc.sync.dma_start(out=l2w_sb[:], in_=lin2w[:])
            nc.sync.dma_start(out=l2b_sb[:], in_=lin2b[:])
            ident = cpool.tile([D, D], bf16, tag="ident")
            make_identity(nc, ident[:])
            emb = cpool.tile([D, gpc], f32, tag="emb")
            eps_sb = cpool.tile([D, 1], f32, tag="eps")
            nc.gpsimd.memset(eps_sb[:], BN_EPS)

            # ---- dis
            dptf = wpool.tile([P, ntile], f32, tag="dptf")
            nc.sync.dma_start(out=dptf[:], in_=deg_pt[:])
            nc.scalar.activation(dis_pt[:], dptf[:],
                                 mybir.ActivationFunctionType.Sqrt, bias=1.0)
            nc.vector.reciprocal(dis_pt[:], dis_pt[:])
            ones1 = cpool.tile([1, D], bf16, tag="ones1")
            nc.gpsimd.memset(ones1[:], 1.0)
            for o in range(0, nsh, RW):
                w = min(RW, nsh - o)
                dsl = wpool.tile([1, RW], f32, tag="dsl")
                nc.sync.dma_start(out=dsl[:, :w], in_=deg_row[:, o:o + w])
                nc.scalar.activation(dsl[:, :w], dsl[:, :w],
                                     mybir.ActivationFunctionType.Sqrt, bias=1.0)
                nc.vector.reciprocal(dsl[:, :w], dsl[:, :w])
                dslb = wpool.tile([1, RW], bf16, tag="dslb")
                nc.vector.tensor_copy(dslb[:, :w], dsl[:, :w])
                pb = ps_z.tile([D, RW], f32, tag="zt", space="PSUM")
                nc.tensor.matmul(pb[:, :w], lhsT=ones1[:], rhs=dslb[:, :w],
                                 start=True, stop=True)
                nc.vector.tensor_copy(dis_rep[:, o:o + w], pb[:, :w])

            # ---- table0 = fp32(x * dis)
            for t in range(ntile):
                w = tsz[t]
                xt = wpool.tile([P, D], f32, tag="xt")
                nc.sync.dma_start(out=xt[:w, :], in_=x_sh[t * P:t * P + w, :])
                xb = wpool.tile([P, D], f32, tag="xb")
                nc.scalar.activation(xb[:w, :], xt[:w, :],
                                     mybir.ActivationFunctionType.Copy,
                                     scale=dis_pt[:w, t:t + 1])
                nc.sync.dma_start(out=tshard[t * P:t * P + w, :], in_=xb[:w, :])
            if "coll" not in ABLATE:
                nc.gpsimd.collective_compute(
                    "AllGather", mybir.AluOpType.bypass,
                    replica_groups=[list(range(N_CORES))],
                    ins=[tshard[:, :].opt()], outs=[table[:, :].opt()])

            # chunk lookup: block col -> chunk index
            chunks = cfg["chunks"]
            n_ch = len(chunks)
            col2chunk = {}
            for k, (base, ncols, _) in enumerate(chunks):
                for cc_ in range(base, base + ncols):
                    col2chunk[cc_] = k

            # ---- layers
            for rep in range(reps):
                for li in range(3):
                    last = (li == 2)
                    Wl = W_sb[li]

                    # lazily-issued gather chunks, one monotone stream
                    chunk_tiles = [None] * n_ch
                    next_issue = [0]
                    qrr = [0]

                    def issue_chunk(k):
                        base, ncols, ih = chunks[k]
                        g = gpool.tile([P, CC, D], f32, tag="g")
                        src_ap = table[HALF:n_nodes, :] if ih \
                            else table[0:HALF, :]
                        # <=1024 idx per call (SWDGE ring limit)
                        for q0 in range(0, ncols, GC):
                            qw = min(GC, ncols - q0)
                            nc.gpsimd.dma_gather(
                                out_ap=g[:, q0:q0 + qw, :], in_ap=src_ap,
                                idxs_ap=idx_res[:, (base + q0) * 8:
                                                (base + q0 + qw) * 8],
                                num_idxs=qw * P, num_idxs_reg=qw * P,
                                elem_size=D,
                                queue_num=1 + qrr[0])
                            qrr[0] = (qrr[0] + 1) % 3
                        gb = gbpool.tile([P, CC, D], bf16, tag="gb")
                        nc.scalar.activation(
                            gb[:, :ncols, :], g[:, :ncols, :],
                            mybir.ActivationFunctionType.Copy)
                        chunk_tiles[k] = (gb, base, ncols)

                    def chunk_of(bcol):
                        k = col2chunk[bcol]
                        while next_issue[0] <= k:
                            issue_chunk(next_issue[0])
                            next_issue[0] += 1
                        return chunk_tiles[k]

                    # O group stream
                    o_tiles = [None] * NG

                    def o_tile(mi):
                        g = mi // OG
                        if o_tiles[g] is None:
                            ot = opool.tile([P, OG, P], fp8, tag="O")
                            # Pool SWDGE (queue 0), NOT nc.sync: every
                            # TileRelease executes on SP, so an SP DMA that
                            # blocks on a pool slot head-of-line blocks the
                            # releases that would free it (deadlock)
                            nc.gpsimd.dma_start(out=ot[:], in_=O_in[g])
                            o_tiles[g] = ot
                        return o_tiles[g]

                    # aggregation + downstream, range-major: one PSUM bank
                    # [64, 512] per range; per tile one accumulation group
                    # (its lo mms then its hi mms), downstream reads the bank
                    for r in range(R):
                        rw = rsz[r]
                        ps = ps_agg.tile([D, RW], f32, tag="agg",
                                         space="PSUM")
                        for t in range(r * 4, min(r * 4 + 4, ntile)):
                            mms = sched0[t]
                            assert mms, "tile with no edges"
                            tw = tsz[t]
                            off = (t - r * 4) * P
                            nmms = len(mms)
                            for j, (mi, b) in enumerate(mms):
                                gb, base, ncols = chunk_of(b)
                                osb = o_tile(mi)
                                nc.tensor.matmul(
                                    ps[:, off:off + tw],
                                    lhsT=gb[:, b - base, :],
                                    rhs=osb[:, mi % OG, :tw],
                                    start=(j == 0), stop=(j == nmms - 1))
                        # downstream per range
                        u2t = wpool.tile([D, RW], bf16, tag="u2t")
                        nc.vector.tensor_tensor(
                            out=u2t[:, :rw], in0=ps[:, :rw],
                            in1=dis_rep[:, r * RW:r * RW + rw],
                            op=mybir.AluOpType.mult)
                        psz = ps_z.tile([D, RW], f32, tag="zt", space="PSUM")
                        nc.tensor.matmul(psz[:, :rw], lhsT=Wl[:],
                                         rhs=u2t[:, :rw],
                                         start=True, stop=True)
                        zslice = (act3 if last else zbuf)[:, r * RW:r * RW + rw]
                        nc.scalar.activation(
                            zslice, psz[:, :rw],
                            mybir.ActivationFunctionType.Copy,
                            accum_out=sums[:, r:r + 1])
                        sq = wpool.tile([D, RW], f32, tag="sq")
                        nc.scalar.activation(
                            sq[:, :rw], psz[:, :rw],
                            mybir.ActivationFunctionType.Square,
                            accum_out=sums2[:, r:r + 1])

                    # ---- global BN stats
                    st = wpool.tile([D, 2], f32, tag="st")
                    nc.vector.reduce_sum(st[:, 0:1], sums[:],
                                         axis=mybir.AxisListType.X)
                    nc.vector.reduce_sum(st[:, 1:2], sums2[:],
                                         axis=mybir.AxisListType.X)
                    nc.sync.dma_start(out=stats_in[:], in_=st[:])
                    if "coll" not in ABLATE:
                        nc.gpsimd.collective_compute(
                            "AllReduce", mybir.AluOpType.add,
                            replica_groups=[list(range(N_CORES))],
                            ins=[stats_in[:, :].opt()],
                            outs=[stats_out[:, :].opt()])
                    stg = wpool.tile([D, 2], f32, tag="stg")
                    nc.sync.dma_start(out=stg[:], in_=stats_out[:])
                    mu = wpool.tile([D, 1], f32, tag="mu")
                    nc.scalar.activation(mu[:], stg[:, 0:1],
                                         mybir.ActivationFunctionType.Copy,
                                         scale=1.0 / n_nodes)
                    va = wpool.tile([D, 1], f32, tag="va")
                    nc.scalar.activation(va[:], stg[:, 1:2],
                                         mybir.ActivationFunctionType.Copy,
                                         scale=1.0 / n_nodes)
                    mu2 = wpool.tile([D, 1], f32, tag="mu2")
                    nc.vector.tensor_tensor(out=mu2[:], in0=mu[:], in1=mu[:],
                                            op=mybir.AluOpType.mult)
                    nc.vector.tensor_tensor(out=va[:], in0=va[:], in1=mu2[:],
                                            op=mybir.AluOpType.subtract)
                    nc.scalar.activation(va[:], va[:],
                                         mybir.ActivationFunctionType.Sqrt,
                                         bias=eps_sb[:])
                    nc.vector.reciprocal(va[:], va[:])
                    saff = wpool.tile([D, 1], f32, tag="saff")
                    nc.vector.tensor_tensor(out=saff[:], in0=gamma_sb[:],
                                            in1=va[:], op=mybir.AluOpType.mult)
                    tsh_ = wpool.tile([D, 1], f32, tag="tsh")
                    nc.vector.tensor_tensor(out=tsh_[:], in0=mu[:], in1=saff[:],
                                            op=mybir.AluOpType.mult)
                    nc.vector.tensor_tensor(out=tsh_[:], in0=beta_sb[:],
                                            in1=tsh_[:],
                                            op=mybir.AluOpType.subtract)

                    # ---- activation phase (per range)
                    for r in range(R):
                        rw = rsz[r]
                        zsl = (act3 if last else zbuf)[:, r * RW:r * RW + rw]
                        at = wpool.tile([D, RW], bf16, tag="at")
                        nc.scalar.activation(at[:, :rw], zsl,
                                             mybir.ActivationFunctionType.Relu,
                                             bias=tsh_[:], scale=saff[:])
                        if not last:
                            ht = wpool.tile([D, RW], bf16, tag="ht")
                            nc.vector.tensor_tensor(
                                out=ht[:, :rw], in0=at[:, :rw],
                                in1=dis_rep[:, r * RW:r * RW + rw],
                                op=mybir.AluOpType.mult)
                            for t in range(r * 4, min(r * 4 + 4, ntile)):
                                w = tsz[t]
                                off = (t - r * 4) * P
                                ptr = ps_tr.tile([P, D], bf16, tag="tr",
                                                 space="PSUM")
                                nc.tensor.transpose(ptr[:w, :],
                                                    ht[:, off:off + w],
                                                    ident[:, :])
                                wr = wpool.tile([P, D], f32, tag="wr")
                                nc.vector.tensor_copy(wr[:w, :], ptr[:w, :])
                                nc.sync.dma_start(
                                    out=tshard[t * P:t * P + w, :],
                                    in_=wr[:w, :])
                        else:
                            nc.vector.tensor_copy(
                                act3[:, r * RW:r * RW + rw], at[:, :rw])
                    if not last and "coll" not in ABLATE:
                        nc.gpsimd.collective_compute(
                            "AllGather", mybir.AluOpType.bypass,
                            replica_groups=[list(range(N_CORES))],
                            ins=[tshard[:, :].opt()],
                            outs=[table[:, :].opt()])

            # ---- pooling
            first_seen = set()
            for (t, s0, s1, g) in cfg["pool_segs"]:
                tmp = wpool.tile([D, 1], f32, tag="ptmp")
                nc.vector.reduce_max(tmp[:], act3[:, t * P + s0:t * P + s1],
                                     axis=mybir.AxisListType.X)
                if g not in first_seen:
                    first_seen.add(g)
                    nc.vector.tensor_copy(emb[:, g:g + 1], tmp[:])
                else:
                    nc.vector.tensor_tensor(out=emb[:, g:g + 1],
                                            in0=emb[:, g:g + 1], in1=tmp[:],
                                            op=mybir.AluOpType.max)

            # ---- head
            emb_bf = wpool.tile([D, gpc], bf16, tag="embbf")
            nc.vector.tensor_copy(emb_bf[:], emb[:])
            ph = ps_z.tile([D, gpc], f32, tag="zt", space="PSUM")
            nc.tensor.matmul(ph[:], lhsT=l1w_sb[:], rhs=emb_bf[:],
                             start=True, stop=True)
            h1 = wpool.tile([D, gpc], bf16, tag="h1")
            nc.scalar.activation(h1[:], ph[:],
                                 mybir.ActivationFunctionType.Relu,
                                 bias=l1b_sb[:])
            po = ps_tr.tile([ncls, gpc], f32, tag="tr", space="PSUM")
            nc.tensor.matmul(po[:], lhsT=l2w_sb[:], rhs=h1[:],
                             start=True, stop=True)
            osb = wpool.tile([ncls, gpc], f32, tag="osb")
            nc.scalar.activation(osb[:], po[:],
                                 mybir.ActivationFunctionType.Identity,
                                 bias=l2b_sb[:])
            nc.sync.dma_start(out=out[:, :].rearrange("g c -> c g"), in_=osb[:])

    nc.compile()
    return nc


# ---------------------------------------------------------------- entry point

_CACHE = {}


def _get_built(cfg_key, cfg, reps):
    key = (cfg_key, reps)
    if key not in _CACHE:
        _CACHE[key] = _build(cfg, reps=reps)
    return _CACHE[key]


def _in_maps(x, data, cfg, W1, W2, W3, gamma, beta,
             lin1_w, lin1_b, lin2_w, lin2_b):
    nsh, ncls = cfg["nsh"], cfg["n_classes"]
    W_bf = [np.asarray(w, np.float32).astype(ml_dtypes.bfloat16)
            for w in (W1, W2, W3)]
    maps = []
    for c in range(N_CORES):
        maps.append({
            "x_sh": x[c * nsh:(c + 1) * nsh].astype(np.float32),
            "idx": data["idx_rep"][c],
            "O": data["Ot"][c],
            "deg_pt": data["deg_pt"][c],
            "deg_row": data["deg_row"][c],
            "W1": W_bf[0], "W2": W_bf[1], "W3": W_bf[2],
            "gamma": np.asarray(gamma, np.float32).reshape(D, 1),
            "beta": np.asarray(beta, np.float32).reshape(D, 1),
            "lin1w": np.asarray(lin1_w, np.float32).astype(ml_dtypes.bfloat16),
            "lin1b": np.asarray(lin1_b, np.float32).reshape(D, 1),
            "lin2w": np.asarray(lin2_w, np.float32).astype(ml_dtypes.bfloat16),
            "lin2b": np.asarray(lin2_b, np.float32).reshape(ncls, 1),
        })
    return maps


def kernel(x, edge_index, batch, W1, b1, W2, b2, W3, b3, gamma, beta,
           lin1_w, lin1_b, lin2_w, lin2_b, _reps=1):
    x = np.asarray(x, np.float32)
    edge_index = np.asarray(edge_index)
    batch = np.asarray(batch)
    n_nodes, d = x.shape
    ncls = np.asarray(lin2_w).shape[1]
    assert d == D

    cfg, data = _prep(x, edge_index, batch, ncls)

    # NOTE: b1/b2/b3 cancel inside BatchNorm (mean subtraction) - unused.
    in_maps = _in_maps(x, data, cfg, W1, W2, W3, gamma, beta,
                       lin1_w, lin1_b, lin2_w, lin2_b)
    cfg_key = (n_nodes, edge_index.shape[1], ncls)
    nc = _get_built(cfg_key, cfg, _reps)
    res = run_bass_kernel_spmd(nc, in_maps, core_ids=list(range(N_CORES)))
    outs = [res.results[c]["out"] for c in range(N_CORES)]
    return np.concatenate(outs, axis=0).astype(np.float32)


# revision 35
# speedup vs baseline: 1.7336x; 1.7336x over previous
"""Trainium2 Bass kernel for nn_GCN1PoolNorm: 3-layer GCN + shared BatchNorm +
global max pool + MLP head, SPMD across 8 NeuronCores.

Self-contained: takes FULL inputs, returns FULL output [N_GRAPHS, N_CLASSES].

Design (per core = one 1/8 dst-shard of nodes) — SPARSE gather + one-hot matmul:
- Node table h_tilde = act * dis lives in Shared DRAM as [n_nodes, 64] fp32
  (256B rows — the dma_gather element granularity), AllGather-published per
  layer.
- Per layer, each core gathers the h_tilde rows of its incident edges' src
  nodes with dma_gather (SWDGE, ~0.34ns/descriptor gen + 256B/desc transfer).
  int16 gather indices cap the table at 32768 rows, so edges are split into a
  lo run (src < 32768) and a hi run (gathered from table[32768:]), each
  dst-sorted and chunked to fit SBUF.
- Aggregation agg[dst] = sum_e h_tilde[src_e] runs as a per-128-edge-block
  matmul: psum[64, tile] += M_block.T @ O_block, where M_block [128, 64] is
  the gathered (bf16-converted) block and O_block [128, 128] fp8 is a
  host-built one-hot edge->dst_local matrix. Exact; PSUM accumulation handles
  duplicate dsts. dis[dst] factors out of the sum and is applied after.
- Self-loops are extra edges (src = dst). lo and hi passes accumulate into an
  SBUF agg buffer (copy, then add).
- Downstream per node tile (as v1): U.T = agg * dis_rep; Z.T = W.T @ U.T;
  BN stats via ACT accum_out; stats AllReduce; BN affine+relu fused; * dis;
  PE transpose; DMA to table shard; AllGather.
- Pooling: graphs align exactly to cores; free-axis reduce_max segments;
  MLP head feat-major; out [gpc, 10] per core, host concatenates.
"""
import numpy as np
import ml_dtypes

from concourse import bacc, mybir, tile
from concourse.bass_utils import run_bass_kernel_spmd
from concourse.masks import make_identity

f32 = mybir.dt.float32
bf16 = mybir.dt.bfloat16
fp8 = mybir.dt.float8e4
i16 = mybir.dt.int16

N_CORES = 8
ABLATE = set()   # sim-only ablation knobs ("coll", "gather", "aggmm")
P = 128          # partition / block quantum
D = 64           # feature dim
HALF = 32768     # int16 gather index limit -> lo/hi table split
CC = 16          # gather chunk columns (CC*128 edge slots per chunk)
GC = 8           # columns per dma_gather call (1024 idx = SWDGE ring limit)
OG = 32          # one-hot matrices per O stream group
BN_EPS = 1e-5


# ---------------------------------------------------------------- host prep

def _prep(x, edge_index, batch, n_classes):
    n_nodes = x.shape[0]
    n_graphs = int(batch.max()) + 1
    assert n_nodes % N_CORES == 0
    nsh = n_nodes // N_CORES                    # nodes per core
    ntile = (nsh + P - 1) // P                  # node tiles per core
    tsz = [min(P, nsh - t * P) for t in range(ntile)]

    src_all = np.asarray(edge_index[0], np.int64)
    dst_all = np.asarray(edge_index[1], np.int64)
    deg = np.bincount(dst_all, minlength=n_nodes).astype(np.int64)

    RW = 512
    RNG = (nsh + RW - 1) // RW                  # 512-node dst ranges per core

    # ---- per-core edge lists, sorted by (dst_range, src_half, dst, src):
    # each (range, half) group is a contiguous slot run so the gather
    # stream is a single monotone sequence of chunks per layer
    edges = []                                  # (s, dl, grp) per core
    for c in range(N_CORES):
        m = (dst_all // nsh) == c
        s = src_all[m]
        dl = dst_all[m] - c * nsh
        # self loops
        s = np.concatenate([s, np.arange(c * nsh, (c + 1) * nsh)])
        dl = np.concatenate([dl, np.arange(nsh)])
        hi = (s >= HALF).astype(np.int64)
        grp = (dl // RW) * 2 + hi
        order = np.lexsort((s, dl, grp))
        s, dl, grp = s[order], dl[order], grp[order]
        edges.append((s, dl, grp))

    # SPMD = one program for all cores: pad every (range, half) group to
    # the max block count over cores; pad slots gather table row 0 of the
    # group's half and are masked by all-zero O columns. Schedules are the
    # union over cores; a core lacking a (group, tile, block) gets an
    # all-zero O matrix (adds 0 to psum).
    NGRP = RNG * 2
    gcols = np.zeros(NGRP, np.int64)            # block columns per group
    for c in range(N_CORES):
        cnt = np.bincount(edges[c][2], minlength=NGRP)
        gcols = np.maximum(gcols, (cnt + P - 1) // P)
    gbase = np.concatenate([[0], np.cumsum(gcols)])  # group -> base col
    SC = int(gbase[-1])

    keys = []
    per_edge = []
    for c in range(N_CORES):
        s, dl, grp = edges[c]
        # slot within group run
        gstart = np.searchsorted(grp, np.arange(NGRP))
        slot = gbase[grp] * P + (np.arange(s.shape[0]) - gstart[grp])
        b = slot // P
        t = dl // P
        hi = grp & 1
        # mm issue order: tile-major, each tile's lo mms then hi mms — the
        # tile's matmuls are one contiguous PSUM accumulation group; chunk
        # consumption steps back at most one range's group span
        key = ((t * 2 + hi) << 24) | b
        keys.append(key)
        per_edge.append((s, dl, slot))
    uk = np.unique(np.concatenate(keys))
    nmm = int(uk.shape[0])
    uk_t = uk >> 25
    uk_b = uk & ((1 << 24) - 1)
    # sched[t] = ordered (mi, b) list (lo blocks then hi blocks)
    sched = [[] for _ in range(ntile)]
    for mi in range(nmm):
        sched[int(uk_t[mi])].append((mi, int(uk_b[mi])))

    # gather chunks: (base_col, ncols, is_hi), never spanning a group
    chunks = []
    for g in range(NGRP):
        for c0 in range(0, int(gcols[g]), CC):
            chunks.append((int(gbase[g]) + c0,
                           int(min(CC, gcols[g] - c0)), g & 1))

    NG = (nmm + OG - 1) // OG
    idx_reps, Ots = [], []
    for c in range(N_CORES):
        s, dl, slot = per_edge[c]
        hi_e = s >= HALF
        # gather index array, 16-wrapped and replicated to 128 partitions;
        # pad slots point at table row 0 (junk, masked by zero O columns)
        idx_flat = np.zeros(SC * P, np.int16)
        idx_flat[slot[~hi_e]] = s[~hi_e].astype(np.int16)
        idx_flat[slot[hi_e]] = (s[hi_e] - HALF).astype(np.int16)
        wrap = idx_flat.reshape(SC * 8, 16).T            # [16, SC*8]
        idx_reps.append(np.tile(wrap, (8, 1)))           # [128, SC*8]

        inv = np.searchsorted(uk, keys[c])
        O = np.zeros((nmm, P, P), np.uint8)
        O[inv, slot % P, dl % P] = 1
        # O stream layout: [NG, 128, OG, 128] fp8, group g col j = O[g*OG+j]
        Ot = np.zeros((NG, P, OG, P), ml_dtypes.float8_e4m3)
        Of = O.astype(ml_dtypes.float8_e4m3)
        for g in range(NG):
            k = min(OG, nmm - g * OG)
            Ot[g, :, :k, :] = Of[g * OG:g * OG + k].transpose(1, 0, 2)
        Ots.append(Ot)
        del O, Of

    # deg layouts (fp32)
    deg_pt = np.zeros((N_CORES, P, ntile), np.float32)
    deg_row = np.zeros((N_CORES, 1, nsh), np.float32)
    for c in range(N_CORES):
        dsh = deg[c * nsh:(c + 1) * nsh].astype(np.float32)
        deg_row[c, 0, :] = dsh
        for tt in range(ntile):
            deg_pt[c, :tsz[tt], tt] = dsh[tt * P:tt * P + tsz[tt]]

    # pooling segments (identical across cores required for SPMD)
    gb = np.searchsorted(batch, np.arange(n_graphs + 1))
    gpc = n_graphs // N_CORES
    loc0 = gb[:gpc + 1].copy()
    for c in range(N_CORES):
        locc = gb[c * gpc:(c + 1) * gpc + 1] - c * nsh
        assert np.array_equal(locc, loc0), "graph pattern must match across cores"
    pool_segs = []
    for tt in range(ntile):
        a, bb = tt * P, tt * P + tsz[tt]
        for g in range(gpc):
            ss, ee = max(a, int(loc0[g])), min(bb, int(loc0[g + 1]))
            if ss < ee:
                pool_segs.append((tt, ss - a, ee - a, g))

    cfg = dict(n_nodes=n_nodes, nsh=nsh, ntile=ntile, tsz=tsz,
               pool_segs=pool_segs, gpc=gpc, n_classes=n_classes,
               n_graphs=n_graphs, SC=SC, NG=NG, chunks=chunks, sched=sched)
    data = dict(idx_rep=idx_reps, Ot=Ots, deg_pt=deg_pt, deg_row=deg_row)
    return cfg, data


# ---------------------------------------------------------------- device build

def _build(cfg, reps=1):
    nsh, ntile, tsz = cfg["nsh"], cfg["ntile"], cfg["tsz"]
    ncls, gpc = cfg["n_classes"], cfg["gpc"]
    n_nodes = cfg["n_nodes"]
    SC, NG = cfg["SC"], cfg["NG"]
    nshp = ntile * P
    RW = 512

    sched0 = cfg["sched"]

    R = (nsh + RW - 1) // RW
    rsz = [min(RW, nsh - r * RW) for r in range(R)]

    nc = bacc.Bacc(trn_type="TRN2", target_bir_lowering=False, debug=False,
                   num_devices=N_CORES, num_swdge_queues=4)

    x_sh = nc.dram_tensor("x_sh", [nsh, D], f32, kind="ExternalInput").ap()
    idx_in = nc.dram_tensor("idx", [P, SC * 8], i16, kind="ExternalInput").ap()
    O_in = nc.dram_tensor("O", [NG, P, OG, P], fp8, kind="ExternalInput").ap()
    deg_pt = nc.dram_tensor("deg_pt", [P, ntile], f32, kind="ExternalInput").ap()
    deg_row = nc.dram_tensor("deg_row", [1, nsh], f32, kind="ExternalInput").ap()
    Ws = [nc.dram_tensor(f"W{i}", [D, D], bf16, kind="ExternalInput").ap()
          for i in (1, 2, 3)]
    gamma = nc.dram_tensor("gamma", [D, 1], f32, kind="ExternalInput").ap()
    beta = nc.dram_tensor("beta", [D, 1], f32, kind="ExternalInput").ap()
    lin1w = nc.dram_tensor("lin1w", [D, D], bf16, kind="ExternalInput").ap()
    lin1b = nc.dram_tensor("lin1b", [D, 1], f32, kind="ExternalInput").ap()
    lin2w = nc.dram_tensor("lin2w", [D, ncls], bf16, kind="ExternalInput").ap()
    lin2b = nc.dram_tensor("lin2b", [ncls, 1], f32, kind="ExternalInput").ap()
    out = nc.dram_tensor("out", [gpc, ncls], f32, kind="ExternalOutput").ap()

    table = nc.dram_tensor("table", [n_nodes, D], f32, addr_space="Shared").ap()
    tshard = nc.dram_tensor("tshard", [nsh, D], f32).ap()
    stats_in = nc.dram_tensor("stats_in", [D, 2], f32).ap()
    stats_out = nc.dram_tensor("stats_out", [D, 2], f32,
                               addr_space="Shared").ap()

    with tile.TileContext(nc) as tc:
        with (
            tc.tile_pool(name="const", bufs=1) as cpool,
            tc.tile_pool(name="gath", bufs=8) as gpool,
            tc.tile_pool(name="gbf", bufs=8) as gbpool,
            tc.tile_pool(name="obuf", bufs=6) as opool,
            tc.tile_pool(name="work", bufs=3) as wpool,
            tc.tile_pool(name="psagg", bufs=3, space="PSUM") as ps_agg,
            tc.tile_pool(name="psz", bufs=2, space="PSUM") as ps_z,
            tc.tile_pool(name="pstr", bufs=2, space="PSUM") as ps_tr,
        ):
            # ---- residents
            idx_res = cpool.tile([P, SC * 8], i16)
            nc.sync.dma_start(out=idx_res[:], in_=idx_in[:])
            dis_pt = cpool.tile([P, ntile], f32)
            dis_rep = cpool.tile([D, nshp], f32)
            zbuf = cpool.tile([D, nshp], bf16)
            act3 = zbuf
            sums = cpool.tile([D, R], f32)
            sums2 = cpool.tile([D, R], f32)
            W_sb = [cpool.tile([D, D], bf16, tag=f"W{i}", name=f"W{i}_sb")
                    for i in range(3)]
            for i in range(3):
                nc.sync.dma_start(out=W_sb[i][:], in_=Ws[i][:])
            gamma_sb = cpool.tile([D, 1], f32, tag="gamma")
            beta_sb = cpool.tile([D, 1], f32, tag="beta")
            nc.sync.dma_start(out=gamma_sb[:], in_=gamma[:])
            nc.sync.dma_start(out=beta_sb[:], in_=beta[:])
            l1w_sb = cpool.tile([D, D], bf16, tag="l1w")
            l1b_sb = cpool.tile([D, 1], f32, tag="l1b")
            l2w_sb = cpool.tile([D, ncls], bf16, tag="l2w")
            l2b_sb = cpool.tile([ncls, 1], f32, tag="l2b")
            nc.sync.dma_start(out=l1w_sb[:], in_=lin1w[:])
            nc.sync.dma_start(out=l1b_sb[:], in_=lin1b[:])
            nc.sync.dma_start(out=l2w_sb[:], in_=lin2w[:])
            nc.sync.dma_start(out=l2b_sb[:], in_=lin2b[:])
            ident = cpool.tile([D, D], bf16, tag="ident")
            make_identity(nc, ident[:])
            emb = cpool.tile([D, gpc], f32, tag="emb")
            eps_sb = cpool.tile([D, 1], f32, tag="eps")
            nc.gpsimd.memset(eps_sb[:], BN_EPS)

            # ---- dis
            dptf = wpool.tile([P, ntile], f32, tag="dptf")
            nc.sync.dma_start(out=dptf[:], in_=deg_pt[:])
            nc.scalar.activation(dis_pt[:], dptf[:],
                                 mybir.ActivationFunctionType.Sqrt, bias=1.0)
            nc.vector.reciprocal(dis_pt[:], dis_pt[:])
            ones1 = cpool.tile([1, D], bf16, tag="ones1")
            nc.gpsimd.memset(ones1[:], 1.0)
            for o in range(0, nsh, RW):
                w = min(RW, nsh - o)
                dsl = wpool.tile([1, RW], f32, tag="dsl")
                nc.sync.dma_start(out=dsl[:, :w], in_=deg_row[:, o:o + w])
                nc.scalar.activation(dsl[:, :w], dsl[:, :w],
                                     mybir.ActivationFunctionType.Sqrt, bias=1.0)
                nc.vector.reciprocal(dsl[:, :w], dsl[:, :w])
                dslb = wpool.tile([1, RW], bf16, tag="dslb")
                nc.vector.tensor_copy(dslb[:, :w], dsl[:, :w])
                pb = ps_z.tile([D, RW], f32, tag="zt", space="PSUM")
                nc.tensor.matmul(pb[:, :w], lhsT=ones1[:], rhs=dslb[:, :w],
                                 start=True, stop=True)
                nc.vector.tensor_copy(dis_rep[:, o:o + w], pb[:, :w])

            # ---- table0 = fp32(x * dis)
            for t in range(ntile):
                w = tsz[t]
                xt = wpool.tile([P, D], f32, tag="xt")
                nc.sync.dma_start(out=xt[:w, :], in_=x_sh[t * P:t * P + w, :])
                xb = wpool.tile([P, D], f32, tag="xb")
                nc.scalar.activation(xb[:w, :], xt[:w, :],
                                     mybir.ActivationFunctionType.Copy,
                                     scale=dis_pt[:w, t:t + 1])
                nc.sync.dma_start(out=tshard[t * P:t * P + w, :], in_=xb[:w, :])
            if "coll" not in ABLATE:
                nc.gpsimd.collective_compute(
                    "AllGather", mybir.AluOpType.bypass,
                    replica_groups=[list(range(N_CORES))],
                    ins=[tshard[:, :].opt()], outs=[table[:, :].opt()])

            # chunk lookup: block col -> chunk index
            chunks = cfg["chunks"]
            n_ch = len(chunks)
            col2chunk = {}
            for k, (base, ncols, _) in enumerate(chunks):
                for cc_ in range(base, base + ncols):
                    col2chunk[cc_] = k

            # ---- layers
            for rep in range(reps):
                for li in range(3):
                    last = (li == 2)
                    Wl = W_sb[li]

                    # lazily-issued gather chunks, one monotone stream
                    chunk_tiles = [None] * n_ch
                    next_issue = [0]
                    qrr = [0]

                    def issue_chunk(k):
                        base, ncols, ih = chunks[k]
                        g = gpool.tile([P, CC, D], f32, tag="g")
                        src_ap = table[HALF:n_nodes, :] if ih \
                            else table[0:HALF, :]
                        # <=1024 idx per call (SWDGE ring limit)
                        for q0 in range(0, ncols, GC):
                            qw = min(GC, ncols - q0)
                            nc.gpsimd.dma_gather(
                                out_ap=g[:, q0:q0 + qw, :], in_ap=src_ap,
                                idxs_ap=idx_res[:, (base + q0) * 8:
                                                (base + q0 + qw) * 8],
                                num_idxs=qw * P, num_idxs_reg=qw * P,
                                elem_size=D,
                                queue_num=qrr[0])
                            qrr[0] = (qrr[0] + 1) % 4
                        gb = gbpool.tile([P, CC, D], bf16, tag="gb")
                        nc.scalar.activation(
                            gb[:, :ncols, :], g[:, :ncols, :],
                            mybir.ActivationFunctionType.Copy)
                        chunk_tiles[k] = (gb, base, ncols)

                    def chunk_of(bcol):
                        k = col2chunk[bcol]
                        while next_issue[0] <= k:
                            issue_chunk(next_issue[0])
                            next_issue[0] += 1
                        return chunk_tiles[k]

                    # O group stream
                    o_tiles = [None] * NG

                    def o_tile(mi):
                        g = mi // OG
                        if o_tiles[g] is None:
                            ot = opool.tile([P, OG, P], fp8, tag="O")
                            nc.sync.dma_start(out=ot[:], in_=O_in[g])
                            o_tiles[g] = ot
                        return o_tiles[g]

                    # aggregation + downstream, range-major: one PSUM bank
                    # [64, 512] per range; per tile one accumulation group
                    # (its lo mms then its hi mms), downstream reads the bank
                    for r in range(R):
                        rw = rsz[r]
                        ps = ps_agg.tile([D, RW], f32, tag="agg",
                                         space="PSUM")
                        for t in range(r * 4, min(r * 4 + 4, ntile)):
                            mms = sched0[t]
                            assert mms, "tile with no edges"
                            tw = tsz[t]
                            off = (t - r * 4) * P
                            nmms = len(mms)
                            for j, (mi, b) in enumerate(mms):
                                gb, base, ncols = chunk_of(b)
                                osb = o_tile(mi)
                                nc.tensor.matmul(
                                    ps[:, off:off + tw],
                                    lhsT=gb[:, b - base, :],
                                    rhs=osb[:, mi % OG, :tw],
                                    start=(j == 0), stop=(j == nmms - 1))
                        # downstream per range
                        u2t = wpool.tile([D, RW], bf16, tag="u2t")
                        nc.vector.tensor_tensor(
                            out=u2t[:, :rw], in0=ps[:, :rw],
                            in1=dis_rep[:, r * RW:r * RW + rw],
                            op=mybir.AluOpType.mult)
                        psz = ps_z.tile([D, RW], f32, tag="zt", space="PSUM")
                        nc.tensor.matmul(psz[:, :rw], lhsT=Wl[:],
                                         rhs=u2t[:, :rw],
                                         start=True, stop=True)
                        zslice = (act3 if last else zbuf)[:, r * RW:r * RW + rw]
                        nc.scalar.activation(
                            zslice, psz[:, :rw],
                            mybir.ActivationFunctionType.Copy,
                            accum_out=sums[:, r:r + 1])
                        sq = wpool.tile([D, RW], f32, tag="sq")
                        nc.scalar.activation(
                            sq[:, :rw], psz[:, :rw],
                            mybir.ActivationFunctionType.Square,
                            accum_out=sums2[:, r:r + 1])

                    # ---- global BN stats
                    st = wpool.tile([D, 2], f32, tag="st")
                    nc.vector.reduce_sum(st[:, 0:1], sums[:],
                                         axis=mybir.AxisListType.X)
                    nc.vector.reduce_sum(st[:, 1:2], sums2[:],
                                         axis=mybir.AxisListType.X)
                    nc.sync.dma_start(out=stats_in[:], in_=st[:])
                    if "coll" not in ABLATE:
                        nc.gpsimd.collective_compute(
                            "AllReduce", mybir.AluOpType.add,
                            replica_groups=[list(range(N_CORES))],
                            ins=[stats_in[:, :].opt()],
                            outs=[stats_out[:, :].opt()])
                    stg = wpool.tile([D, 2], f32, tag="stg")
                    nc.sync.dma_start(out=stg[:], in_=stats_out[:])
                    mu = wpool.tile([D, 1], f32, tag="mu")
                    nc.scalar.activation(mu[:], stg[:, 0:1],
                                         mybir.ActivationFunctionType.Copy,
                                         scale=1.0 / n_nodes)
                    va = wpool.tile([D, 1], f32, tag="va")
                    nc.scalar.activation(va[:], stg[:, 1:2],
                                         mybir.ActivationFunctionType.Copy,
                                         scale=1.0 / n_nodes)
                    mu2 = wpool.tile([D, 1], f32, tag="mu2")
                    nc.vector.tensor_tensor(out=mu2[:], in0=mu[:], in1=mu[:],
                                            op=mybir.AluOpType.mult)
                    nc.vector.tensor_tensor(out=va[:], in0=va[:], in1=mu2[:],
                                            op=mybir.AluOpType.subtract)
                    nc.scalar.activation(va[:], va[:],
                                         mybir.ActivationFunctionType.Sqrt,
                                         bias=eps_sb[:])
                    nc.vector.reciprocal(va[:], va[:])
                    saff = wpool.tile([D, 1], f32, tag="saff")
                    nc.vector.tensor_tensor(out=saff[:], in0=gamma_sb[:],
                                            in1=va[:], op=mybir.AluOpType.mult)
                    tsh_ = wpool.tile([D, 1], f32, tag="tsh")
                    nc.vector.tensor_tensor(out=tsh_[:], in0=mu[:], in1=saff[:],
                                            op=mybir.AluOpType.mult)
                    nc.vector.tensor_tensor(out=tsh_[:], in0=beta_sb[:],
                                            in1=tsh_[:],
                                            op=mybir.AluOpType.subtract)

                    # ---- activation phase (per range)
                    for r in range(R):
                        rw = rsz[r]
                        zsl = (act3 if last else zbuf)[:, r * RW:r * RW + rw]
                        at = wpool.tile([D, RW], bf16, tag="at")
                        nc.scalar.activation(at[:, :rw], zsl,
                                             mybir.ActivationFunctionType.Relu,
                                             bias=tsh_[:], scale=saff[:])
                        if not last:
                            ht = wpool.tile([D, RW], bf16, tag="ht")
                            nc.vector.tensor_tensor(
                                out=ht[:, :rw], in0=at[:, :rw],
                                in1=dis_rep[:, r * RW:r * RW + rw],
                                op=mybir.AluOpType.mult)
                            for t in range(r * 4, min(r * 4 + 4, ntile)):
                                w = tsz[t]
                                off = (t - r * 4) * P
                                ptr = ps_tr.tile([P, D], bf16, tag="tr",
                                                 space="PSUM")
                                nc.tensor.transpose(ptr[:w, :],
                                                    ht[:, off:off + w],
                                                    ident[:, :])
                                wr = wpool.tile([P, D], f32, tag="wr")
                                nc.vector.tensor_copy(wr[:w, :], ptr[:w, :])
                                nc.sync.dma_start(
                                    out=tshard[t * P:t * P + w, :],
                                    in_=wr[:w, :])
                        else:
                            nc.vector.tensor_copy(
                                act3[:, r * RW:r * RW + rw], at[:, :rw])
                    if not last and "coll" not in ABLATE:
                        nc.gpsimd.collective_compute(
                            "AllGather", mybir.AluOpType.bypass,
                            replica_groups=[list(range(N_CORES))],
                            ins=[tshard[:, :].opt()],
                            outs=[table[:, :].opt()])

            # ---- pooling
            first_seen = set()
            for (t, s0, s1, g) in cfg["pool_segs"]:
                tmp = wpool.tile([D, 1], f32, tag="ptmp")
                nc.vector.reduce_max(tmp[:], act3[:, t * P + s0:t * P + s1],
                                     axis=mybir.AxisListType.X)
                if g not in first_seen:
                    first_seen.add(g)
                    nc.vector.tensor_copy(emb[:, g:g + 1], tmp[:])
                else:
                    nc.vector.tensor_tensor(out=emb[:, g:g + 1],
                                            in0=emb[:, g:g + 1], in1=tmp[:],
                                            op=mybir.AluOpType.max)

            # ---- head
            emb_bf = wpool.tile([D, gpc], bf16, tag="embbf")
            nc.vector.tensor_copy(emb_bf[:], emb[:])
            ph = ps_z.tile([D, gpc], f32, tag="zt", space="PSUM")
            nc.tensor.matmul(ph[:], lhsT=l1w_sb[:], rhs=emb_bf[:],
                             start=True, stop=True)
            h1 = wpool.tile([D, gpc], bf16, tag="h1")
            nc.scalar.activation(h1[:], ph[:],
                                 mybir.ActivationFunctionType.Relu,
                                 bias=l1b_sb[:])
            po = ps_tr.tile([ncls, gpc], f32, tag="tr", space="PSUM")
            nc.tensor.matmul(po[:], lhsT=l2w_sb[:], rhs=h1[:],
                             start=True, stop=True)
            osb = wpool.tile([ncls, gpc], f32, tag="osb")
            nc.scalar.activation(osb[:], po[:],
                                 mybir.ActivationFunctionType.Identity,
                                 bias=l2b_sb[:])
            nc.sync.dma_start(out=out[:, :].rearrange("g c -> c g"), in_=osb[:])

    nc.compile()
    return nc


# ---------------------------------------------------------------- entry point

_CACHE = {}


def _get_built(cfg_key, cfg, reps):
    key = (cfg_key, reps)
    if key not in _CACHE:
        _CACHE[key] = _build(cfg, reps=reps)
    return _CACHE[key]


def _in_maps(x, data, cfg, W1, W2, W3, gamma, beta,
             lin1_w, lin1_b, lin2_w, lin2_b):
    nsh, ncls = cfg["nsh"], cfg["n_classes"]
    W_bf = [np.asarray(w, np.float32).astype(ml_dtypes.bfloat16)
            for w in (W1, W2, W3)]
    maps = []
    for c in range(N_CORES):
        maps.append({
            "x_sh": x[c * nsh:(c + 1) * nsh].astype(np.float32),
            "idx": data["idx_rep"][c],
            "O": data["Ot"][c],
            "deg_pt": data["deg_pt"][c],
            "deg_row": data["deg_row"][c],
            "W1": W_bf[0], "W2": W_bf[1], "W3": W_bf[2],
            "gamma": np.asarray(gamma, np.float32).reshape(D, 1),
            "beta": np.asarray(beta, np.float32).reshape(D, 1),
            "lin1w": np.asarray(lin1_w, np.float32).astype(ml_dtypes.bfloat16),
            "lin1b": np.asarray(lin1_b, np.float32).reshape(D, 1),
            "lin2w": np.asarray(lin2_w, np.float32).astype(ml_dtypes.bfloat16),
            "lin2b": np.asarray(lin2_b, np.float32).reshape(ncls, 1),
        })
    return maps


def kernel(x, edge_index, batch, W1, b1, W2, b2, W3, b3, gamma, beta,
           lin1_w, lin1_b, lin2_w, lin2_b, _reps=1):
    x = np.asarray(x, np.float32)
    edge_index = np.asarray(edge_index)
    batch = np.asarray(batch)
    n_nodes, d = x.shape
    ncls = np.asarray(lin2_w).shape[1]
    assert d == D

    cfg, data = _prep(x, edge_index, batch, ncls)

    # NOTE: b1/b2/b3 cancel inside BatchNorm (mean subtraction) - unused.
    in_maps = _in_maps(x, data, cfg, W1, W2, W3, gamma, beta,
                       lin1_w, lin1_b, lin2_w, lin2_b)
    cfg_key = (n_nodes, edge_index.shape[1], ncls)
    nc = _get_built(cfg_key, cfg, _reps)
    res = run_bass_kernel_spmd(nc, in_maps, core_ids=list(range(N_CORES)))
    outs = [res.results[c]["out"] for c in range(N_CORES)]
    return np.concatenate(outs, axis=0).astype(np.float32)


# revision 40
# speedup vs baseline: 1.7717x; 1.0220x over previous
"""Trainium2 Bass kernel for nn_GCN1PoolNorm: 3-layer GCN + shared BatchNorm +
global max pool + MLP head, SPMD across 8 NeuronCores.

Self-contained: takes FULL inputs, returns FULL output [N_GRAPHS, N_CLASSES].

Design (per core = one 1/8 dst-shard of nodes) — SPARSE gather + one-hot matmul:
- Node table h_tilde = act * dis lives in Shared DRAM as [n_nodes, 64] fp32
  (256B rows — the dma_gather element granularity), AllGather-published per
  layer.
- Per layer, each core gathers the h_tilde rows of its incident edges' src
  nodes with dma_gather (max 1024 idx per call = SWDGE ring limit; calls
  round-robin the 4 SWDGE queues so ring drains overlap desc generation).
  int16 gather indices cap the table at 32768 rows, so edges are split into
  lo (src < 32768) and hi groups per 512-dst range, sorted by
  (dst_range, src_half, dst) so the gather stream is one monotone chunk
  sequence.
- Aggregation agg[dst] = sum_e h_tilde[src_e] runs range-major: one PSUM
  bank [64, 512] per dst range; per node tile one contiguous accumulation
  group of matmuls psum[:, tile] += M_block.T @ O_block, where M_block
  [128, 64] is the gathered (bf16-converted) 128-edge block and O_block
  [128, 128] fp8 is a host-built one-hot edge->dst_local matrix. Exact;
  PSUM accumulation handles duplicate dsts; dis[dst] factors out of the
  sum. Self-loops are extra edges (src = dst).
- Downstream per range reads the bank: U.T = psum * dis_rep; Z.T = W.T@U.T;
  BN stats via ACT accum_out; stats AllReduce; BN affine+relu fused; * dis;
  PE transpose per tile; DMA to table shard; AllGather.
- Pooling: graphs align exactly to cores; free-axis reduce_max segments;
  MLP head feat-major; out [gpc, 10] per core, host concatenates.
"""
import numpy as np
import ml_dtypes

from concourse import bacc, mybir, tile
from concourse.bass_utils import run_bass_kernel_spmd
from concourse.masks import make_identity

f32 = mybir.dt.float32
bf16 = mybir.dt.bfloat16
fp8 = mybir.dt.float8e4
i16 = mybir.dt.int16

N_CORES = 8
ABLATE = set()   # sim-only ablation knob ("coll")
P = 128          # partition / block quantum
D = 64           # feature dim
HALF = 32768     # int16 gather index limit -> lo/hi table split
CC = 16          # gather chunk columns (CC*128 edge slots per chunk)
GC = 8           # columns per dma_gather call (1024 idx = SWDGE ring limit)
OG = 32          # one-hot matrices per O stream group
BN_EPS = 1e-5


# ---------------------------------------------------------------- host prep

def _prep(x, edge_index, batch, n_classes):
    n_nodes = x.shape[0]
    n_graphs = int(batch.max()) + 1
    assert n_nodes % N_CORES == 0
    nsh = n_nodes // N_CORES                    # nodes per core
    ntile = (nsh + P - 1) // P                  # node tiles per core
    tsz = [min(P, nsh - t * P) for t in range(ntile)]

    src_all = np.asarray(edge_index[0], np.int64)
    dst_all = np.asarray(edge_index[1], np.int64)
    deg = np.bincount(dst_all, minlength=n_nodes).astype(np.int64)

    RW = 512
    RNG = (nsh + RW - 1) // RW                  # 512-node dst ranges per core

    # ---- per-core edge lists, sorted by (dst_range, src_half, dst, src):
    # each (range, half) group is a contiguous slot run so the gather
    # stream is a single monotone sequence of chunks per layer
    edges = []                                  # (s, dl, grp) per core
    for c in range(N_CORES):
        m = (dst_all // nsh) == c
        s = src_all[m]
        dl = dst_all[m] - c * nsh
        # self loops
        s = np.concatenate([s, np.arange(c * nsh, (c + 1) * nsh)])
        dl = np.concatenate([dl, np.arange(nsh)])
        hi = (s >= HALF).astype(np.int64)
        grp = (dl // RW) * 2 + hi
        order = np.lexsort((s, dl, grp))
        s, dl, grp = s[order], dl[order], grp[order]
        edges.append((s, dl, grp))

    # SPMD = one program for all cores: pad every (range, half) group to
    # the max block count over cores; pad slots gather table row 0 of the
    # group's half and are masked by all-zero O columns. Schedules are the
    # union over cores; a core lacking a (group, tile, block) gets an
    # all-zero O matrix (adds 0 to psum).
    NGRP = RNG * 2
    gcols = np.zeros(NGRP, np.int64)            # block columns per group
    for c in range(N_CORES):
        cnt = np.bincount(edges[c][2], minlength=NGRP)
        gcols = np.maximum(gcols, (cnt + P - 1) // P)
    gbase = np.concatenate([[0], np.cumsum(gcols)])  # group -> base col
    SC = int(gbase[-1])

    keys = []
    per_edge = []
    for c in range(N_CORES):
        s, dl, grp = edges[c]
        # slot within group run
        gstart = np.searchsorted(grp, np.arange(NGRP))
        slot = gbase[grp] * P + (np.arange(s.shape[0]) - gstart[grp])
        b = slot // P
        t = dl // P
        hi = grp & 1
        # mm issue order: tile-major, each tile's lo mms then hi mms — the
        # tile's matmuls are one contiguous PSUM accumulation group; chunk
        # consumption steps back at most one range's group span
        key = ((t * 2 + hi) << 24) | b
        keys.append(key)
        per_edge.append((s, dl, slot))
    uk = np.unique(np.concatenate(keys))
    nmm = int(uk.shape[0])
    uk_t = uk >> 25
    uk_b = uk & ((1 << 24) - 1)
    # sched[t] = ordered (mi, b) list (lo blocks then hi blocks)
    sched = [[] for _ in range(ntile)]
    for mi in range(nmm):
        sched[int(uk_t[mi])].append((mi, int(uk_b[mi])))

    # gather chunks: (base_col, ncols, is_hi), never spanning a group
    chunks = []
    for g in range(NGRP):
        for c0 in range(0, int(gcols[g]), CC):
            chunks.append((int(gbase[g]) + c0,
                           int(min(CC, gcols[g] - c0)), g & 1))

    NG = (nmm + OG - 1) // OG
    idx_reps, Ots = [], []
    for c in range(N_CORES):
        s, dl, slot = per_edge[c]
        hi_e = s >= HALF
        # gather index array, 16-wrapped and replicated to 128 partitions;
        # pad slots point at table row 0 (junk, masked by zero O columns)
        idx_flat = np.zeros(SC * P, np.int16)
        idx_flat[slot[~hi_e]] = s[~hi_e].astype(np.int16)
        idx_flat[slot[hi_e]] = (s[hi_e] - HALF).astype(np.int16)
        wrap = idx_flat.reshape(SC * 8, 16).T            # [16, SC*8]
        idx_reps.append(np.tile(wrap, (8, 1)))           # [128, SC*8]

        inv = np.searchsorted(uk, keys[c])
        O = np.zeros((nmm, P, P), np.uint8)
        O[inv, slot % P, dl % P] = 1
        # O stream layout: [NG, 128, OG, 128] fp8, group g col j = O[g*OG+j]
        Ot = np.zeros((NG, P, OG, P), ml_dtypes.float8_e4m3)
        Of = O.astype(ml_dtypes.float8_e4m3)
        for g in range(NG):
            k = min(OG, nmm - g * OG)
            Ot[g, :, :k, :] = Of[g * OG:g * OG + k].transpose(1, 0, 2)
        Ots.append(Ot)
        del O, Of

    # deg layouts (fp32)
    deg_pt = np.zeros((N_CORES, P, ntile), np.float32)
    deg_row = np.zeros((N_CORES, 1, nsh), np.float32)
    for c in range(N_CORES):
        dsh = deg[c * nsh:(c + 1) * nsh].astype(np.float32)
        deg_row[c, 0, :] = dsh
        for tt in range(ntile):
            deg_pt[c, :tsz[tt], tt] = dsh[tt * P:tt * P + tsz[tt]]

    # pooling segments (identical across cores required for SPMD)
    gb = np.searchsorted(batch, np.arange(n_graphs + 1))
    gpc = n_graphs // N_CORES
    loc0 = gb[:gpc + 1].copy()
    for c in range(N_CORES):
        locc = gb[c * gpc:(c + 1) * gpc + 1] - c * nsh
        assert np.array_equal(locc, loc0), "graph pattern must match across cores"
    pool_segs = []
    for tt in range(ntile):
        a, bb = tt * P, tt * P + tsz[tt]
        for g in range(gpc):
            ss, ee = max(a, int(loc0[g])), min(bb, int(loc0[g + 1]))
            if ss < ee:
                pool_segs.append((tt, ss - a, ee - a, g))

    cfg = dict(n_nodes=n_nodes, nsh=nsh, ntile=ntile, tsz=tsz,
               pool_segs=pool_segs, gpc=gpc, n_classes=n_classes,
               n_graphs=n_graphs, SC=SC, NG=NG, chunks=chunks, sched=sched)
    data = dict(idx_rep=idx_reps, Ot=Ots, deg_pt=deg_pt, deg_row=deg_row)
    return cfg, data


# ---------------------------------------------------------------- device build

def _build(cfg, reps=1):
    nsh, ntile, tsz = cfg["nsh"], cfg["ntile"], cfg["tsz"]
    ncls, gpc = cfg["n_classes"], cfg["gpc"]
    n_nodes = cfg["n_nodes"]
    SC, NG = cfg["SC"], cfg["NG"]
    nshp = ntile * P
    RW = 512

    sched0 = cfg["sched"]

    R = (nsh + RW - 1) // RW
    rsz = [min(RW, nsh - r * RW) for r in range(R)]

    nc = bacc.Bacc(trn_type="TRN2", target_bir_lowering=False, debug=False,
                   num_devices=N_CORES, num_swdge_queues=4)

    x_sh = nc.dram_tensor("x_sh", [nsh, D], f32, kind="ExternalInput").ap()
    idx_in = nc.dram_tensor("idx", [P, SC * 8], i16, kind="ExternalInput").ap()
    O_in = nc.dram_tensor("O", [NG, P, OG, P], fp8, kind="ExternalInput").ap()
    deg_pt = nc.dram_tensor("deg_pt", [P, ntile], f32, kind="ExternalInput").ap()
    deg_row = nc.dram_tensor("deg_row", [1, nsh], f32, kind="ExternalInput").ap()
    Ws = [nc.dram_tensor(f"W{i}", [D, D], bf16, kind="ExternalInput").ap()
          for i in (1, 2, 3)]
    gamma = nc.dram_tensor("gamma", [D, 1], f32, kind="ExternalInput").ap()
    beta = nc.dram_tensor("beta", [D, 1], f32, kind="ExternalInput").ap()
    lin1w = nc.dram_tensor("lin1w", [D, D], bf16, kind="ExternalInput").ap()
    lin1b = nc.dram_tensor("lin1b", [D, 1], f32, kind="ExternalInput").ap()
    lin2w = nc.dram_tensor("lin2w", [D, ncls], bf16, kind="ExternalInput").ap()
    lin2b = nc.dram_tensor("lin2b", [ncls, 1], f32, kind="ExternalInput").ap()
    out = nc.dram_tensor("out", [gpc, ncls], f32, kind="ExternalOutput").ap()

    table = nc.dram_tensor("table", [n_nodes, D], f32, addr_space="Shared").ap()
    tshard = nc.dram_tensor("tshard", [nsh, D], f32).ap()
    stats_in = nc.dram_tensor("stats_in", [D, 2], f32).ap()
    stats_out = nc.dram_tensor("stats_out", [D, 2], f32,
                               addr_space="Shared").ap()

    with tile.TileContext(nc) as tc:
        with (
            tc.tile_pool(name="const", bufs=1) as cpool,
            tc.tile_pool(name="gath", bufs=8) as gpool,
            tc.tile_pool(name="gbf", bufs=8) as gbpool,
            tc.tile_pool(name="obuf", bufs=6) as opool,
            tc.tile_pool(name="work", bufs=3) as wpool,
            tc.tile_pool(name="psagg", bufs=3, space="PSUM") as ps_agg,
            tc.tile_pool(name="psz", bufs=2, space="PSUM") as ps_z,
            tc.tile_pool(name="pstr", bufs=2, space="PSUM") as ps_tr,
        ):
            # ---- residents
            idx_res = cpool.tile([P, SC * 8], i16)
            nc.sync.dma_start(out=idx_res[:], in_=idx_in[:])
            dis_pt = cpool.tile([P, ntile], f32)
            dis_rep = cpool.tile([D, nshp], f32)
            zbuf = cpool.tile([D, nshp], bf16)
            act3 = zbuf
            sums = cpool.tile([D, R], f32)
            sums2 = cpool.tile([D, R], f32)
            W_sb = [cpool.tile([D, D], bf16, tag=f"W{i}", name=f"W{i}_sb")
                    for i in range(3)]
            for i in range(3):
                nc.sync.dma_start(out=W_sb[i][:], in_=Ws[i][:])
            gamma_sb = cpool.tile([D, 1], f32, tag="gamma")
            beta_sb = cpool.tile([D, 1], f32, tag="beta")
            nc.sync.dma_start(out=gamma_sb[:], in_=gamma[:])
            nc.sync.dma_start(out=beta_sb[:], in_=beta[:])
            l1w_sb = cpool.tile([D, D], bf16, tag="l1w")
            l1b_sb = cpool.tile([D, 1], f32, tag="l1b")
            l2w_sb = cpool.tile([D, ncls], bf16, tag="l2w")
            l2b_sb = cpool.tile([ncls, 1], f32, tag="l2b")
            nc.sync.dma_start(out=l1w_sb[:], in_=lin1w[:])
            nc.sync.dma_start(out=l1b_sb[:], in_=lin1b[:])
            nc.sync.dma_start(out=l2w_sb[:], in_=lin2w[:])
            nc.sync.dma_start(out=l2b_sb[:], in_=lin2b[:])
            ident = cpool.tile([D, D], bf16, tag="ident")
            make_identity(nc, ident[:])
            emb = cpool.tile([D, gpc], f32, tag="emb")
            eps_sb = cpool.tile([D, 1], f32, tag="eps")
            nc.gpsimd.memset(eps_sb[:], BN_EPS)

            # ---- dis
            dptf = wpool.tile([P, ntile], f32, tag="dptf")
            nc.sync.dma_start(out=dptf[:], in_=deg_pt[:])
            nc.scalar.activation(dis_pt[:], dptf[:],
                                 mybir.ActivationFunctionType.Sqrt, bias=1.0)
            nc.vector.reciprocal(dis_pt[:], dis_pt[:])
            ones1 = cpool.tile([1, D], bf16, tag="ones1")
            nc.gpsimd.memset(ones1[:], 1.0)
            for o in range(0, nsh, RW):
                w = min(RW, nsh - o)
                dsl = wpool.tile([1, RW], f32, tag="dsl")
                nc.sync.dma_start(out=dsl[:, :w], in_=deg_row[:, o:o + w])
                nc.scalar.activation(dsl[:, :w], dsl[:, :w],
                                     mybir.ActivationFunctionType.Sqrt, bias=1.0)
                nc.vector.reciprocal(dsl[:, :w], dsl[:, :w])
                dslb = wpool.tile([1, RW], bf16, tag="dslb")
                nc.vector.tensor_copy(dslb[:, :w], dsl[:, :w])
                pb = ps_z.tile([D, RW], f32, tag="zt", space="PSUM")
                nc.tensor.matmul(pb[:, :w], lhsT=ones1[:], rhs=dslb[:, :w],
                                 start=True, stop=True)
                nc.vector.tensor_copy(dis_rep[:, o:o + w], pb[:, :w])

            # ---- table0 = fp32(x * dis)
            for t in range(ntile):
                w = tsz[t]
                xt = wpool.tile([P, D], f32, tag="xt")
                nc.sync.dma_start(out=xt[:w, :], in_=x_sh[t * P:t * P + w, :])
                xb = wpool.tile([P, D], f32, tag="xb")
                nc.scalar.activation(xb[:w, :], xt[:w, :],
                                     mybir.ActivationFunctionType.Copy,
                                     scale=dis_pt[:w, t:t + 1])
                nc.sync.dma_start(out=tshard[t * P:t * P + w, :], in_=xb[:w, :])
            if "coll" not in ABLATE:
                nc.gpsimd.collective_compute(
                    "AllGather", mybir.AluOpType.bypass,
                    replica_groups=[list(range(N_CORES))],
                    ins=[tshard[:, :].opt()], outs=[table[:, :].opt()])

            # chunk lookup: block col -> chunk index
            chunks = cfg["chunks"]
            n_ch = len(chunks)
            col2chunk = {}
            for k, (base, ncols, _) in enumerate(chunks):
                for cc_ in range(base, base + ncols):
                    col2chunk[cc_] = k

            # ---- layers
            for rep in range(reps):
                for li in range(3):
                    last = (li == 2)
                    Wl = W_sb[li]

                    # lazily-issued gather chunks, one monotone stream
                    chunk_tiles = [None] * n_ch
                    next_issue = [0]
                    qrr = [0]

                    def issue_chunk(k):
                        base, ncols, ih = chunks[k]
                        g = gpool.tile([P, CC, D], f32, tag="g")
                        src_ap = table[HALF:n_nodes, :] if ih \
                            else table[0:HALF, :]
                        # <=1024 idx per call (SWDGE ring limit)
                        for q0 in range(0, ncols, GC):
                            qw = min(GC, ncols - q0)
                            nc.gpsimd.dma_gather(
                                out_ap=g[:, q0:q0 + qw, :], in_ap=src_ap,
                                idxs_ap=idx_res[:, (base + q0) * 8:
                                                (base + q0 + qw) * 8],
                                num_idxs=qw * P, num_idxs_reg=qw * P,
                                elem_size=D,
                                queue_num=qrr[0])
                            qrr[0] = (qrr[0] + 1) % 4
                        gb = gbpool.tile([P, CC, D], bf16, tag="gb")
                        nc.scalar.activation(
                            gb[:, :ncols, :], g[:, :ncols, :],
                            mybir.ActivationFunctionType.Copy)
                        chunk_tiles[k] = (gb, base, ncols)

                    def chunk_of(bcol):
                        k = col2chunk[bcol]
                        while next_issue[0] <= k:
                            issue_chunk(next_issue[0])
                            next_issue[0] += 1
                        return chunk_tiles[k]

                    # O group stream
                    o_tiles = [None] * NG

                    def o_tile(mi):
                        g = mi // OG
                        if o_tiles[g] is None:
                            ot = opool.tile([P, OG, P], fp8, tag="O")
                            nc.sync.dma_start(out=ot[:], in_=O_in[g])
                            o_tiles[g] = ot
                        return o_tiles[g]

                    # aggregation + downstream, range-major: one PSUM bank
                    # [64, 512] per range; per tile one accumulation group
                    # (its lo mms then its hi mms), downstream reads the bank
                    def downstream(r, ps):
                        rw = rsz[r]
                        u2t = wpool.tile([D, RW], bf16, tag="u2t")
                        nc.vector.tensor_tensor(
                            out=u2t[:, :rw], in0=ps[:, :rw],
                            in1=dis_rep[:, r * RW:r * RW + rw],
                            op=mybir.AluOpType.mult)
                        psz = ps_z.tile([D, RW], f32, tag="zt", space="PSUM")
                        nc.tensor.matmul(psz[:, :rw], lhsT=Wl[:],
                                         rhs=u2t[:, :rw],
                                         start=True, stop=True)
                        zslice = (act3 if last else zbuf)[:, r * RW:r * RW + rw]
                        nc.scalar.activation(
                            zslice, psz[:, :rw],
                            mybir.ActivationFunctionType.Copy,
                            accum_out=sums[:, r:r + 1])
                        sq = wpool.tile([D, RW], f32, tag="sq")
                        nc.scalar.activation(
                            sq[:, :rw], psz[:, :rw],
                            mybir.ActivationFunctionType.Square,
                            accum_out=sums2[:, r:r + 1])

                    for r in range(R):
                        rw = rsz[r]
                        ps = ps_agg.tile([D, RW], f32, tag="agg",
                                         space="PSUM")
                        for t in range(r * 4, min(r * 4 + 4, ntile)):
                            mms = sched0[t]
                            assert mms, "tile with no edges"
                            tw = tsz[t]
                            off = (t - r * 4) * P
                            nmms = len(mms)
                            for j, (mi, b) in enumerate(mms):
                                gb, base, ncols = chunk_of(b)
                                osb = o_tile(mi)
                                nc.tensor.matmul(
                                    ps[:, off:off + tw],
                                    lhsT=gb[:, b - base, :],
                                    rhs=osb[:, mi % OG, :tw],
                                    start=(j == 0), stop=(j == nmms - 1))
                        downstream(r, ps)

                    # ---- global BN stats
                    st = wpool.tile([D, 2], f32, tag="st")
                    nc.vector.reduce_sum(st[:, 0:1], sums[:],
                                         axis=mybir.AxisListType.X)
                    nc.vector.reduce_sum(st[:, 1:2], sums2[:],
                                         axis=mybir.AxisListType.X)
                    nc.sync.dma_start(out=stats_in[:], in_=st[:])
                    if "coll" not in ABLATE:
                        nc.gpsimd.collective_compute(
                            "AllReduce", mybir.AluOpType.add,
                            replica_groups=[list(range(N_CORES))],
                            ins=[stats_in[:, :].opt()],
                            outs=[stats_out[:, :].opt()])
                    stg = wpool.tile([D, 2], f32, tag="stg")
                    nc.sync.dma_start(out=stg[:], in_=stats_out[:])
                    mu = wpool.tile([D, 1], f32, tag="mu")
                    nc.scalar.activation(mu[:], stg[:, 0:1],
                                         mybir.ActivationFunctionType.Copy,
                                         scale=1.0 / n_nodes)
                    va = wpool.tile([D, 1], f32, tag="va")
                    nc.scalar.activation(va[:], stg[:, 1:2],
                                         mybir.ActivationFunctionType.Copy,
                                         scale=1.0 / n_nodes)
                    mu2 = wpool.tile([D, 1], f32, tag="mu2")
                    nc.vector.tensor_tensor(out=mu2[:], in0=mu[:], in1=mu[:],
                                            op=mybir.AluOpType.mult)
                    nc.vector.tensor_tensor(out=va[:], in0=va[:], in1=mu2[:],
                                            op=mybir.AluOpType.subtract)
                    nc.scalar.activation(va[:], va[:],
                                         mybir.ActivationFunctionType.Sqrt,
                                         bias=eps_sb[:])
                    nc.vector.reciprocal(va[:], va[:])
                    saff = wpool.tile([D, 1], f32, tag="saff")
                    nc.vector.tensor_tensor(out=saff[:], in0=gamma_sb[:],
                                            in1=va[:], op=mybir.AluOpType.mult)
                    tsh_ = wpool.tile([D, 1], f32, tag="tsh")
                    nc.vector.tensor_tensor(out=tsh_[:], in0=mu[:], in1=saff[:],
                                            op=mybir.AluOpType.mult)
                    nc.vector.tensor_tensor(out=tsh_[:], in0=beta_sb[:],
                                            in1=tsh_[:],
                                            op=mybir.AluOpType.subtract)

                    # ---- activation phase (per range)
                    for r in range(R):
                        rw = rsz[r]
                        zsl = (act3 if last else zbuf)[:, r * RW:r * RW + rw]
                        at = wpool.tile([D, RW], bf16, tag="at")
                        nc.scalar.activation(at[:, :rw], zsl,
                                             mybir.ActivationFunctionType.Relu,
                                             bias=tsh_[:], scale=saff[:])
                        if not last:
                            ht = wpool.tile([D, RW], bf16, tag="ht")
                            nc.vector.tensor_tensor(
                                out=ht[:, :rw], in0=at[:, :rw],
                                in1=dis_rep[:, r * RW:r * RW + rw],
                                op=mybir.AluOpType.mult)
                            for t in range(r * 4, min(r * 4 + 4, ntile)):
                                w = tsz[t]
                                off = (t - r * 4) * P
                                ptr = ps_tr.tile([P, D], bf16, tag="tr",
                                                 space="PSUM")
                                nc.tensor.transpose(ptr[:w, :],
                                                    ht[:, off:off + w],
                                                    ident[:, :])
                                wr = wpool.tile([P, D], f32, tag="wr")
                                nc.vector.tensor_copy(wr[:w, :], ptr[:w, :])
                                nc.sync.dma_start(
                                    out=tshard[t * P:t * P + w, :],
                                    in_=wr[:w, :])
                        else:
                            nc.vector.tensor_copy(
                                act3[:, r * RW:r * RW + rw], at[:, :rw])
                    if not last and "coll" not in ABLATE:
                        nc.gpsimd.collective_compute(
                            "AllGather", mybir.AluOpType.bypass,
                            replica_groups=[list(range(N_CORES))],
                            ins=[tshard[:, :].opt()],
                            outs=[table[:, :].opt()])

            # ---- pooling
            first_seen = set()
            for (t, s0, s1, g) in cfg["pool_segs"]:
                tmp = wpool.tile([D, 1], f32, tag="ptmp")
                nc.vector.reduce_max(tmp[:], act3[:, t * P + s0:t * P + s1],
                                     axis=mybir.AxisListType.X)
                if g not in first_seen:
                    first_seen.add(g)
                    nc.vector.tensor_copy(emb[:, g:g + 1], tmp[:])
                else:
                    nc.vector.tensor_tensor(out=emb[:, g:g + 1],
                                            in0=emb[:, g:g + 1], in1=tmp[:],
                                            op=mybir.AluOpType.max)

            # ---- head
            emb_bf = wpool.tile([D, gpc], bf16, tag="embbf")
            nc.vector.tensor_copy(emb_bf[:], emb[:])
            ph = ps_z.tile([D, gpc], f32, tag="zt", space="PSUM")
            nc.tensor.matmul(ph[:], lhsT=l1w_sb[:], rhs=emb_bf[:],
                             start=True, stop=True)
            h1 = wpool.tile([D, gpc], bf16, tag="h1")
            nc.scalar.activation(h1[:], ph[:],
                                 mybir.ActivationFunctionType.Relu,
                                 bias=l1b_sb[:])
            po = ps_tr.tile([ncls, gpc], f32, tag="tr", space="PSUM")
            nc.tensor.matmul(po[:], lhsT=l2w_sb[:], rhs=h1[:],
                             start=True, stop=True)
            osb = wpool.tile([ncls, gpc], f32, tag="osb")
            nc.scalar.activation(osb[:], po[:],
                                 mybir.ActivationFunctionType.Identity,
                                 bias=l2b_sb[:])
            nc.sync.dma_start(out=out[:, :].rearrange("g c -> c g"), in_=osb[:])

    nc.compile()
    return nc


# ---------------------------------------------------------------- entry point

_CACHE = {}


def _get_built(cfg_key, cfg, reps):
    key = (cfg_key, reps)
    if key not in _CACHE:
        _CACHE[key] = _build(cfg, reps=reps)
    return _CACHE[key]


def _in_maps(x, data, cfg, W1, W2, W3, gamma, beta,
             lin1_w, lin1_b, lin2_w, lin2_b):
    nsh, ncls = cfg["nsh"], cfg["n_classes"]
    W_bf = [np.asarray(w, np.float32).astype(ml_dtypes.bfloat16)
            for w in (W1, W2, W3)]
    maps = []
    for c in range(N_CORES):
        maps.append({
            "x_sh": x[c * nsh:(c + 1) * nsh].astype(np.float32),
            "idx": data["idx_rep"][c],
            "O": data["Ot"][c],
            "deg_pt": data["deg_pt"][c],
            "deg_row": data["deg_row"][c],
            "W1": W_bf[0], "W2": W_bf[1], "W3": W_bf[2],
            "gamma": np.asarray(gamma, np.float32).reshape(D, 1),
            "beta": np.asarray(beta, np.float32).reshape(D, 1),
            "lin1w": np.asarray(lin1_w, np.float32).astype(ml_dtypes.bfloat16),
            "lin1b": np.asarray(lin1_b, np.float32).reshape(D, 1),
            "lin2w": np.asarray(lin2_w, np.float32).astype(ml_dtypes.bfloat16),
            "lin2b": np.asarray(lin2_b, np.float32).reshape(ncls, 1),
        })
    return maps


def kernel(x, edge_index, batch, W1, b1, W2, b2, W3, b3, gamma, beta,
           lin1_w, lin1_b, lin2_w, lin2_b, _reps=1):
    x = np.asarray(x, np.float32)
    edge_index = np.asarray(edge_index)
    batch = np.asarray(batch)
    n_nodes, d = x.shape
    ncls = np.asarray(lin2_w).shape[1]
    assert d == D

    cfg, data = _prep(x, edge_index, batch, ncls)

    # NOTE: b1/b2/b3 cancel inside BatchNorm (mean subtraction) - unused.
    in_maps = _in_maps(x, data, cfg, W1, W2, W3, gamma, beta,
                       lin1_w, lin1_b, lin2_w, lin2_b)
    cfg_key = (n_nodes, edge_index.shape[1], ncls)
    nc = _get_built(cfg_key, cfg, _reps)
    res = run_bass_kernel_spmd(nc, in_maps, core_ids=list(range(N_CORES)))
    outs = [res.results[c]["out"] for c in range(N_CORES)]
    return np.concatenate(outs, axis=0).astype(np.float32)
